# revision 1
# baseline (speedup 1.0000x reference)
"""Trainium2 Bass kernel for nn_BaseTransformer (ensemble member-attention block).

Sharding: data-parallel over batch B=8 across 8 NeuronCores (1 batch each).
Weights/constants replicated. No collectives.

Reference math (per batch b, x = in_tensor[b] as [K=16, C=64, S=4096]):
  value = einsum('ics,oc->ios', x, Wv)
  key   = selu(einsum(x, Wk)); query = selu(einsum(x, Wq))
  gram[c,i,j] = sum_s key[i,c,s] query[j,c,s] / 64        (then * lambda^2 fold)
  A = softmax(gram, axis=i) + I
  transformed[j] = sum_i (A[c,i,j] - 1/16) value_i        (exact mean fold)
  out = selu(x + einsum(transformed, w_out) + b_out)

Layout/dtype scheme (v2):
  - x_bf16 resident as 8 pair tiles [128, S] (members t, t+8); feeds the
    transposed k/q conv (x chunks as PE stationary operand -> k,q come out
    [s, heads]) and the value conv. x_fp32 is re-streamed from HBM in phase 2
    only for the exact residual add.
  - stride-8 head groups (head c = 8u+g) so gram operands are single-stride
    APs and the value gather/scatter DMAs use contiguous partition runs
    (sigma = bit-swap permutation folded into Wv columns / Wout rows).
  - selu(t) = min(alpha*e^t - alpha, relu(t)) composed exactly from
    ACT Exp (bias ln a), ACT Relu, DVE scalar_tensor_tensor (sub/min).
  - mix matmul is block-diagonal over 8 heads x 16 members with the
    B matrices assembled via permutation matmuls (P^T (softmax masked)^T P').
"""

import sys

if "/opt/trn_rl_repo" not in sys.path:
    sys.path.insert(0, "/opt/trn_rl_repo")

import numpy as np

import concourse.bass as bass
import concourse.bacc as bacc
import concourse.mybir as mybir
import concourse.tile as tile

F32 = mybir.dt.float32
BF16 = mybir.dt.bfloat16

K, C, HEADS, S = 16, 64, 64, 4096
NG = 8           # head groups of 8 (stride-8: group g = heads {8u+g})
SC1 = 128        # phase-1 spatial chunk (gram contraction tile)
NCH1 = S // SC1  # 32
SC2 = 512        # phase-2 spatial chunk
NCH2 = S // SC2  # 8

ALPHA = 1.6732632423543772
LAMBDA = 1.0507009873554805
LN_ALPHA = float(np.log(ALPHA))
LN_LAMBDA_ALPHA = float(np.log(LAMBDA * ALPHA))
GRAM_SCALE = float(LAMBDA * LAMBDA / 64.0)


def _pi(u, i):
    return 64 * (i // 8) + 8 * u + (i % 8)


def host_constants(w_value, w_key, w_query, w_out, b_out):
    """Build all replicated device inputs on the host."""
    consts = {}
    # sigma: head c = 8u+g  <->  storage position 8g+u (group-contiguous).
    sigma = np.zeros(64, np.int64)
    for u in range(8):
        for g in range(8):
            sigma[8 * g + u] = 8 * u + g
    wvT = np.ascontiguousarray(w_value.T[:, sigma])
    consts["wvT"] = np.concatenate([wvT, wvT], axis=0).astype(np.float32)
    wkqT = np.ascontiguousarray(np.concatenate([w_key.T, w_query.T], axis=1))
    consts["wkqT"] = np.concatenate([wkqT, wkqT], axis=0).astype(np.float32)
    woutT = np.ascontiguousarray(w_out.T[sigma, :])
    consts["woutT"] = np.concatenate([woutT, woutT], axis=0).astype(np.float32)

    # Gram psum layout: partition = 8j+u (q side), free = 8i+u' (k side).
    # MASK zeroes cross-head entries (u != u').
    mask = np.zeros((128, 128), np.float32)
    for p in range(128):
        for f in range(128):
            if p % 8 == f % 8:
                mask[p, f] = 1.0
    consts["maskg"] = mask

    # P (mm2 lhsT): rows r=(i,u)=8i+u -> out partition pi(u, i); same matrix
    # serves as P' (mm1 rhs) for the j side.
    P = np.zeros((128, 128), np.float32)
    for u in range(8):
        for i in range(16):
            P[8 * i + u, _pi(u, i)] = 1.0
    consts["permP"] = P
    consts["permPp"] = P.copy()

    # DPAT in permuted coords: D[pi(u,i), pi(u,j)] = delta(i,j) - 1/16.
    D = np.zeros((128, 128), np.float32)
    for u in range(8):
        for i in range(16):
            for j in range(16):
                D[_pi(u, i), _pi(u, j)] = (1.0 if i == j else 0.0) - 1.0 / 16.0
    consts["dpat"] = D

    consts["b_out_col"] = np.concatenate([b_out, b_out]).astype(
        np.float32).reshape(128, 1)
    return consts


def build_nc():
    """Build the single-core Bass program (same NEFF on all 8 cores)."""
    nc = bacc.Bacc("TRN2", target_bir_lowering=False, debug=False)

    x_d = nc.dram_tensor("x", [K, C, S], F32, kind="ExternalInput")
    wvT_d = nc.dram_tensor("wvT", [128, 64], F32, kind="ExternalInput")
    wkqT_d = nc.dram_tensor("wkqT", [128, 128], F32, kind="ExternalInput")
    woutT_d = nc.dram_tensor("woutT", [128, 64], F32, kind="ExternalInput")
    mask_d = nc.dram_tensor("maskg", [128, 128], F32, kind="ExternalInput")
    permP_d = nc.dram_tensor("permP", [128, 128], F32, kind="ExternalInput")
    permPp_d = nc.dram_tensor("permPp", [128, 128], F32, kind="ExternalInput")
    dpat_d = nc.dram_tensor("dpat", [128, 128], F32, kind="ExternalInput")
    bo_d = nc.dram_tensor("b_out_col", [128, 1], F32, kind="ExternalInput")
    out_d = nc.dram_tensor("out", [K, C, S], F32, kind="ExternalOutput")

    with tile.TileContext(nc) as tc:
        with (
            tc.tile_pool(name="persist", bufs=1) as persist,
            tc.tile_pool(name="xpool", bufs=1) as xpool,
        ):
            # ---- weights / constants to SBUF (+ bf16 casts) ----
            wv_f = persist.tile([128, 64], F32, tag="wvf")
            nc.sync.dma_start(out=wv_f, in_=wvT_d[:, :])
            wv_sb = persist.tile([128, 64], BF16, tag="wv")
            nc.gpsimd.tensor_copy(wv_sb, wv_f)
            wkq_f = persist.tile([128, 128], F32, tag="wkqf")
            nc.sync.dma_start(out=wkq_f, in_=wkqT_d[:, :])
            wkq_sb = persist.tile([128, 128], BF16, tag="wkq")
            nc.gpsimd.tensor_copy(wkq_sb, wkq_f)
            wo_f = persist.tile([128, 64], F32, tag="wof")
            nc.sync.dma_start(out=wo_f, in_=woutT_d[:, :])
            wo_sb = persist.tile([128, 64], BF16, tag="wo")
            nc.gpsimd.tensor_copy(wo_sb, wo_f)
            mask_sb = persist.tile([128, 128], F32, tag="mask")
            nc.sync.dma_start(out=mask_sb, in_=mask_d[:, :])
            permP_sb = persist.tile([128, 128], F32, tag="permP")
            nc.sync.dma_start(out=permP_sb, in_=permP_d[:, :])
            permPp_sb = persist.tile([128, 128], F32, tag="permPp")
            nc.sync.dma_start(out=permPp_sb, in_=permPp_d[:, :])
            dpat_sb = persist.tile([128, 128], F32, tag="dpat")
            nc.sync.dma_start(out=dpat_sb, in_=dpat_d[:, :])
            bo_sb = persist.tile([128, 1], F32, tag="bo")
            nc.sync.dma_start(out=bo_sb, in_=bo_d[:, :])
            lna_sb = persist.tile([128, 1], F32, tag="lna")
            nc.vector.memset(lna_sb, LN_ALPHA)
            lnla_sb = persist.tile([128, 1], F32, tag="lnla")
            nc.vector.memset(lnla_sb, LN_LAMBDA_ALPHA)
            zero_sb = persist.tile([128, 1], F32, tag="zero")
            nc.vector.memset(zero_sb, 0.0)

            # ---- x: stream fp32, cast to resident bf16 pair tiles ----
            x_sb = []
            with tc.tile_pool(name="xload", bufs=3) as xload:
                for t in range(8):
                    xf = xload.tile([128, S], F32, tag="xf")
                    nc.sync.dma_start(out=xf[0:64, :], in_=x_d[t, :, :])
                    nc.sync.dma_start(out=xf[64:128, :], in_=x_d[t + 8, :, :])
                    xb = xpool.tile([128, S], BF16, tag=f"x{t}")
                    if t % 3 == 1:
                        nc.scalar.copy(xb, xf)
                    elif t % 3 == 2:
                        nc.gpsimd.tensor_copy(xb, xf)
                    else:
                        nc.vector.tensor_copy(xb, xf)
                    x_sb.append(xb)

            # BigB result tiles (persist into phase 2), bf16 for the mix matmul
            bigB = []
            for g in range(NG):
                bigB_t = persist.tile([128, 128], BF16, tag=f"bigB{g}")
                bigB.append(bigB_t)

            # =========================== PHASE 1 ===========================
            with (
                tc.tile_pool(name="p1sb", bufs=3) as p1sb,
                tc.tile_pool(name="p1sc", bufs=3) as p1sc,
                tc.tile_pool(name="kqps", bufs=2, space="PSUM") as kqps,
                tc.tile_pool(name="gramps", bufs=1, space="PSUM") as gramps,
            ):
                gram_ps = []
                for gb in range(2):
                    gram_t = gramps.tile([128, 512], F32, tag=f"gram{gb}")
                    gram_ps.append(gram_t)

                for sc in range(NCH1):
                    sl = slice(SC1 * sc, SC1 * (sc + 1))
                    # kqT free layout: half*1024 + m*64 + c   (bf16)
                    kqT = p1sb.tile([128, K * 128], BF16, tag="kqT")
                    for blk in range(2):  # member blocks [0..8), [8..16)
                        ps = kqps.tile([128, 8 * 128], F32, tag="kqps")
                        for mb in range(8):
                            m = blk * 8 + mb
                            xt = x_sb[m % 8]
                            rhalf = slice(0, 64) if m < 8 else slice(64, 128)
                            nc.tensor.matmul(
                                ps[:, 128 * mb: 128 * (mb + 1)],
                                xt[rhalf, sl], wkq_sb[rhalf, :],
                                start=True, stop=True,
                            )
                        # selu: e2 = exp(kq + ln a); r = relu(kq);
                        # out = (e2 - a) min r   (all bf16 outputs).
                        # e2/r stored half-split (h, mb, c) so the stt reads
                        # contiguous halves (DVE 2x bf16 mode).
                        e2 = p1sc.tile([128, 8 * 128], BF16, tag="e2")
                        e2v = e2.rearrange("p (h mb c) -> p mb h c",
                                           mb=8, h=2, c=64)
                        nc.scalar.activation(
                            out=e2v, in_=ps,
                            func=mybir.ActivationFunctionType.Exp,
                            bias=lna_sb[:, 0:1])
                        r = p1sc.tile([128, 8 * 128], BF16, tag="r")
                        rv = r.rearrange("p (h mb c) -> p mb h c",
                                         mb=8, h=2, c=64)
                        if (sc + blk) % 2 == 0:
                            nc.scalar.activation(
                                out=rv, in_=ps,
                                func=mybir.ActivationFunctionType.Relu,
                                bias=zero_sb[:, 0:1])
                        else:
                            nc.vector.tensor_scalar(
                                out=rv, in0=ps, scalar1=0.0, scalar2=None,
                                op0=mybir.AluOpType.max)
                        for half in range(2):
                            nc.vector.scalar_tensor_tensor(
                                out=kqT[:, 1024 * half + 512 * blk:
                                        1024 * half + 512 * (blk + 1)],
                                in0=e2[:, 512 * half: 512 * (half + 1)],
                                scalar=ALPHA,
                                in1=r[:, 512 * half: 512 * (half + 1)],
                                op0=mybir.AluOpType.subtract,
                                op1=mybir.AluOpType.min)
                    # gram: lhsT = q side (M = 8j+u), rhs = k side (N = 8i+u'),
                    # single-stride [[8,128]] APs at offset g
                    vq = kqT.rearrange("p (f e) -> p e f", f=256, e=8)
                    for g in range(NG):
                        q_ap = vq[:, g, 128:256]
                        k_ap = vq[:, g, 0:128]
                        nc.tensor.matmul(
                            gram_ps[g // 4][:, 128 * (g % 4): 128 * (g % 4 + 1)],
                            q_ap, k_ap,
                            start=(sc == 0 and g % 4 == 0),
                            stop=(sc == NCH1 - 1 and g % 4 == 3))

                # ---- softmax (no max-sub; range pre-verified) + BigB ----
                for g in range(NG):
                    gp = gram_ps[g // 4][:, 128 * (g % 4): 128 * (g % 4 + 1)]
                    E = p1sc.tile([128, 128], F32, tag="E")
                    nc.scalar.activation(
                        out=E, in_=gp,
                        func=mybir.ActivationFunctionType.Exp,
                        bias=zero_sb[:, 0:1], scale=GRAM_SCALE)
                    Ssum = p1sc.tile([128, 8], F32, tag="Ssum")
                    nc.vector.tensor_reduce(
                        out=Ssum,
                        in_=E.rearrange("p (i u) -> p u i", i=16, u=8),
                        axis=mybir.AxisListType.X, op=mybir.AluOpType.add)
                    R = p1sc.tile([128, 8], F32, tag="R")
                    nc.vector.reciprocal(out=R, in_=Ssum)
                    Eu = E.rearrange("p (i u) -> p u i", i=16, u=8)
                    for u in range(8):
                        nc.vector.tensor_scalar(
                            out=Eu[:, u, :], in0=Eu[:, u, :],
                            scalar1=R[:, u: u + 1], scalar2=None,
                            op0=mybir.AluOpType.mult)
                    nc.vector.tensor_tensor(
                        out=E, in0=E, in1=mask_sb, op=mybir.AluOpType.mult)
                    c_ps = kqps.tile([128, 128], F32, tag="kqps")
                    nc.tensor.matmul(c_ps, E, permPp_sb, start=True, stop=True)
                    c_sb = p1sc.tile([128, 128], F32, tag="permcsb")
                    nc.scalar.copy(c_sb, c_ps)
                    b_ps = kqps.tile([128, 128], F32, tag="kqps")
                    nc.tensor.matmul(b_ps, permP_sb, c_sb, start=True, stop=True)
                    nc.vector.scalar_tensor_tensor(
                        out=bigB[g], in0=b_ps, scalar=1.0, in1=dpat_sb,
                        op0=mybir.AluOpType.mult, op1=mybir.AluOpType.add)

            # =========================== PHASE 2 ===========================
            # SBUF->SBUF DMAs lower to SP-serial DIRECT2D (slow), so the
            # partition-regrouping shuffles round-trip through DRAM scratch:
            # DRAM-involved DMAs ride the fast DGE path and DRAM-side access
            # patterns may stride arbitrarily. Scratch is per-chunk (no WAR).
            # Software-pipelined 3 stages: value(pc) | mix(pc-1) | out(pc-2).
            # vscr[pc][g][64*i2 + 8u + it][s] -- gather g is a plain 2D load.
            # mscr[pc][jt][64*j2 + 8g + u][s] -- tload jt is a plain 2D load.
            vscr_d = nc.dram_tensor("vscr", [NCH2, NG, 128, SC2], BF16)
            mscr_d = nc.dram_tensor("mscr", [NCH2, 8, 128, SC2], BF16)
            with (
                tc.tile_pool(name="xsp", bufs=4) as xsp,
                tc.tile_pool(name="p2sc", bufs=6) as p2sc,
                tc.tile_pool(name="p2out", bufs=3) as p2outp,
                tc.tile_pool(name="vps", bufs=3, space="PSUM") as vps,
                tc.tile_pool(name="mps", bufs=3, space="PSUM") as mps,
                tc.tile_pool(name="ops", bufs=2, space="PSUM") as ops,
            ):
                xv_d = x_d.rearrange("(m2 mt) c s -> mt m2 c s", m2=2, mt=8)
                ov_d = out_d.rearrange("(m2 mt) c s -> mt m2 c s", m2=2, mt=8)
                vstores = {}
                mstores = {}

                def stage_value(pc):
                    sl = slice(SC2 * pc, SC2 * (pc + 1))
                    vdst = vscr_d[pc].rearrange(
                        "g (i2 u it) s -> it i2 g u s", i2=2, u=8, it=8)
                    stores = []
                    for t in range(8):
                        ps = vps.tile([128, SC2], F32, tag="vps")
                        nc.tensor.matmul(
                            ps[0:64, :], wv_sb[0:64, :], x_sb[t][0:64, sl],
                            start=True, stop=True)
                        nc.tensor.matmul(
                            ps[64:128, :], wv_sb[64:128, :], x_sb[t][64:128, sl],
                            start=True, stop=True)
                        vpair = p2sc.tile([128, SC2], BF16, tag="vpair")
                        nc.vector.tensor_copy(vpair, ps)
                        for i2 in range(2):
                            sti = nc.sync.dma_start(
                                out=vdst[t, i2],
                                in_=vpair[64 * i2: 64 * (i2 + 1), :])
                            stores.append(sti)
                    vstores[pc] = stores

                def stage_mix(pc):
                    stores = vstores.pop(pc)
                    mdst = mscr_d[pc].rearrange(
                        "jt (j2 gg u) s -> gg j2 u jt s", j2=2, gg=8, u=8)
                    mst = []
                    for g in range(NG):
                        pm = mps.tile([128, SC2], F32, tag="mps")
                        vg = p2sc.tile([128, SC2], BF16, tag="vg")
                        gi = nc.scalar.dma_start(out=vg, in_=vscr_d[pc, g])
                        for sti in stores:
                            tile.add_dep_helper(
                                gi.ins, sti.ins, reason="gather after vstores")
                        nc.tensor.matmul(pm, bigB[g], vg, start=True, stop=True)
                        mg = p2sc.tile([128, SC2], BF16, tag="mg")
                        nc.scalar.copy(mg, pm)
                        for j2 in range(2):
                            si = nc.sync.dma_start(
                                out=mdst[g, j2],
                                in_=mg[64 * j2: 64 * (j2 + 1), :])
                            mst.append(si)
                    mstores[pc] = mst

                def stage_out(pc):
                    sl = slice(SC2 * pc, SC2 * (pc + 1))
                    mst = mstores.pop(pc)
                    for jt in range(8):
                        xs = xsp.tile([128, SC2], F32, tag="xs")
                        nc.sync.dma_start(out=xs, in_=xv_d[jt, :, :, sl])
                        tpair = p2sc.tile([128, SC2], BF16, tag="tpair")
                        li = nc.scalar.dma_start(out=tpair, in_=mscr_d[pc, jt])
                        for si in mst:
                            tile.add_dep_helper(
                                li.ins, si.ins, reason="tload after mstores")
                        po = ops.tile([128, SC2], F32, tag="ops")
                        nc.tensor.matmul(
                            po[0:64, :], wo_sb[0:64, :], tpair[0:64, :],
                            start=True, stop=True)
                        nc.tensor.matmul(
                            po[64:128, :], wo_sb[64:128, :], tpair[64:128, :],
                            start=True, stop=True)
                        # y = po + b_out + x   (exact fp32 residual)
                        ty = p2sc.tile([128, SC2], F32, tag="ty")
                        nc.vector.scalar_tensor_tensor(
                            out=ty, in0=po, scalar=bo_sb[:, 0:1], in1=xs,
                            op0=mybir.AluOpType.add, op1=mybir.AluOpType.add)
                        # selu(y) = min(l*a*e^y - l*a, l*relu(y))
                        e2f = p2sc.tile([128, SC2], F32, tag="fe2")
                        nc.scalar.activation(
                            out=e2f, in_=ty,
                            func=mybir.ActivationFunctionType.Exp,
                            bias=lnla_sb[:, 0:1])
                        r2f = p2sc.tile([128, SC2], F32, tag="fr2")
                        nc.vector.tensor_scalar(
                            out=r2f, in0=ty, scalar1=0.0, scalar2=LAMBDA,
                            op0=mybir.AluOpType.max, op1=mybir.AluOpType.mult)
                        o_sb = p2outp.tile([128, SC2], F32, tag="osb")
                        nc.vector.scalar_tensor_tensor(
                            out=o_sb, in0=e2f, scalar=float(LAMBDA * ALPHA),
                            in1=r2f,
                            op0=mybir.AluOpType.subtract,
                            op1=mybir.AluOpType.min)
                        nc.sync.dma_start(out=ov_d[jt, :, :, sl], in_=o_sb)

                for pc in range(NCH2 + 2):
                    if pc < NCH2:
                        stage_value(pc)
                    if 1 <= pc <= NCH2:
                        stage_mix(pc - 1)
                    if pc >= 2:
                        stage_out(pc - 2)
    nc.compile()
    return nc


_NC_CACHE = None


def _get_nc():
    global _NC_CACHE
    if _NC_CACHE is None:
        _NC_CACHE = build_nc()
    return _NC_CACHE


def kernel(in_tensor, w_value, w_key, w_query, w_out, b_out, **_ignored):
    in_tensor = np.asarray(in_tensor, dtype=np.float32)
    w_value = np.asarray(w_value, dtype=np.float32)
    w_key = np.asarray(w_key, dtype=np.float32)
    w_query = np.asarray(w_query, dtype=np.float32)
    w_out = np.asarray(w_out, dtype=np.float32)
    b_out = np.asarray(b_out, dtype=np.float32)

    B = in_tensor.shape[0]
    assert B == 8
    consts = host_constants(w_value, w_key, w_query, w_out, b_out)

    nc = _get_nc()
    in_maps = []
    for b in range(B):
        m = {"x": np.ascontiguousarray(in_tensor[b].reshape(K, C, S))}
        m.update(consts)
        in_maps.append(m)

    from concourse.bass_utils import run_bass_kernel_spmd

    res = run_bass_kernel_spmd(nc, in_maps, core_ids=list(range(8)))
    outs = [res.results[b]["out"].reshape(K, C, 64, 64) for b in range(B)]
    return np.stack(outs, axis=0).astype(np.float32)


if __name__ == "__main__":
    build_nc()
    print("built ok")



# revision 8
# speedup vs baseline: 1.9703x; 1.9703x over previous
"""Trainium2 Bass kernel for nn_BaseTransformer (ensemble member-attention block).

Sharding: data-parallel over batch B=8 across 8 NeuronCores (1 batch each).
Weights/constants replicated. No collectives.

Reference math (per batch b, x = in_tensor[b] as [K=16, C=64, S=4096]):
  value = einsum('ics,oc->ios', x, Wv)
  key   = selu(einsum(x, Wk)); query = selu(einsum(x, Wq))
  gram[c,i,j] = sum_s key[i,c,s] query[j,c,s] / 64        (then * lambda^2 fold)
  A = softmax(gram, axis=i) + I
  transformed[j] = sum_i (A[c,i,j] - 1/16) value_i        (exact mean fold)
  out = selu(x + einsum(transformed, w_out) + b_out)

Layout/dtype scheme (v2):
  - x_bf16 resident as 8 pair tiles [128, S] (members t, t+8); feeds the
    transposed k/q conv (x chunks as PE stationary operand -> k,q come out
    [s, heads]) and the value conv. x_fp32 is re-streamed from HBM in phase 2
    only for the exact residual add.
  - stride-8 head groups (head c = 8u+g) so gram operands are single-stride
    APs and the value gather/scatter DMAs use contiguous partition runs
    (sigma = bit-swap permutation folded into Wv columns / Wout rows).
  - selu(t) = min(alpha*e^t - alpha, relu(t)) composed exactly from
    ACT Exp (bias ln a), ACT Relu, DVE scalar_tensor_tensor (sub/min).
  - mix matmul is block-diagonal over 8 heads x 16 members with the
    B matrices assembled via permutation matmuls (P^T (softmax masked)^T P').
"""

import sys

if "/opt/trn_rl_repo" not in sys.path:
    sys.path.insert(0, "/opt/trn_rl_repo")

import numpy as np

import concourse.bass as bass
import concourse.bacc as bacc
import concourse.mybir as mybir
import concourse.tile as tile

F32 = mybir.dt.float32
BF16 = mybir.dt.bfloat16

K, C, HEADS, S = 16, 64, 64, 4096
NG = 8           # head groups of 8 (stride-8: group g = heads {8u+g})
SC1 = 128        # phase-1 spatial chunk (gram contraction tile)
NCH1 = S // SC1  # 32
SC2 = 512        # phase-2 spatial chunk
NCH2 = S // SC2  # 8

ALPHA = 1.6732632423543772
LAMBDA = 1.0507009873554805
LN_ALPHA = float(np.log(ALPHA))
LN_LAMBDA_ALPHA = float(np.log(LAMBDA * ALPHA))
GRAM_SCALE = float(LAMBDA * LAMBDA / 64.0)


def _pi(u, i):
    return 64 * (i // 8) + 8 * u + (i % 8)


def host_constants(w_value, w_key, w_query, w_out, b_out):
    """Build all replicated device inputs on the host."""
    consts = {}
    # sigma: head c = 8u+g  <->  storage position 8g+u (group-contiguous).
    sigma = np.zeros(64, np.int64)
    for u in range(8):
        for g in range(8):
            sigma[8 * g + u] = 8 * u + g
    wvT = np.ascontiguousarray(w_value.T[:, sigma])
    consts["wvT"] = np.concatenate([wvT, wvT], axis=0).astype(np.float32)
    wkqT = np.ascontiguousarray(np.concatenate([w_key.T, w_query.T], axis=1))
    consts["wkqT"] = np.concatenate([wkqT, wkqT], axis=0).astype(np.float32)
    woutT = np.ascontiguousarray(w_out.T[sigma, :])
    consts["woutT"] = np.concatenate([woutT, woutT], axis=0).astype(np.float32)

    # Gram psum layout: partition = 8j+u (q side), free = 8i+u' (k side).
    # MASK zeroes cross-head entries (u != u').
    mask = np.zeros((128, 128), np.float32)
    for p in range(128):
        for f in range(128):
            if p % 8 == f % 8:
                mask[p, f] = 1.0
    consts["maskg"] = mask

    # P (mm2 lhsT): rows r=(i,u)=8i+u -> out partition pi(u, i); same matrix
    # serves as P' (mm1 rhs) for the j side.
    P = np.zeros((128, 128), np.float32)
    for u in range(8):
        for i in range(16):
            P[8 * i + u, _pi(u, i)] = 1.0
    consts["permP"] = P
    consts["permPp"] = P.copy()

    # DPAT in permuted coords: D[pi(u,i), pi(u,j)] = delta(i,j) - 1/16.
    D = np.zeros((128, 128), np.float32)
    for u in range(8):
        for i in range(16):
            for j in range(16):
                D[_pi(u, i), _pi(u, j)] = (1.0 if i == j else 0.0) - 1.0 / 16.0
    consts["dpat"] = D

    consts["b_out_col"] = np.concatenate([b_out, b_out]).astype(
        np.float32).reshape(128, 1)
    return consts


def build_nc():
    """Build the single-core Bass program (same NEFF on all 8 cores)."""
    nc = bacc.Bacc("TRN2", target_bir_lowering=False, debug=False)

    x_d = nc.dram_tensor("x", [K, C, S], F32, kind="ExternalInput")
    # out layout [jt, pc, (m2 c), s]: every store is one fully-contiguous
    # 256KB DRAM block (outer DRAM dim = 128 partitions -> stripes across all
    # 16 SDMA engines; the old [K,C,S]-strided stores collapsed onto 2).
    # Host inverse-permutes.
    wvT_d = nc.dram_tensor("wvT", [128, 64], F32, kind="ExternalInput")
    wkqT_d = nc.dram_tensor("wkqT", [128, 128], F32, kind="ExternalInput")
    woutT_d = nc.dram_tensor("woutT", [128, 64], F32, kind="ExternalInput")
    mask_d = nc.dram_tensor("maskg", [128, 128], F32, kind="ExternalInput")
    permP_d = nc.dram_tensor("permP", [128, 128], F32, kind="ExternalInput")
    permPp_d = nc.dram_tensor("permPp", [128, 128], F32, kind="ExternalInput")
    dpat_d = nc.dram_tensor("dpat", [128, 128], F32, kind="ExternalInput")
    bo_d = nc.dram_tensor("b_out_col", [128, 1], F32, kind="ExternalInput")
    out_d = nc.dram_tensor("out", [8, NCH2, 128, SC2], F32,
                           kind="ExternalOutput")

    with tile.TileContext(nc) as tc:
        with (
            tc.tile_pool(name="persist", bufs=1) as persist,
            tc.tile_pool(name="xpool", bufs=1) as xpool,
        ):
            # ---- weights / constants to SBUF (+ bf16 casts) ----
            wv_f = persist.tile([128, 64], F32, tag="wvf")
            nc.sync.dma_start(out=wv_f, in_=wvT_d[:, :])
            wv_sb = persist.tile([128, 64], BF16, tag="wv")
            nc.gpsimd.tensor_copy(wv_sb, wv_f)
            wkq_f = persist.tile([128, 128], F32, tag="wkqf")
            nc.sync.dma_start(out=wkq_f, in_=wkqT_d[:, :])
            wkq_sb = persist.tile([128, 128], BF16, tag="wkq")
            nc.gpsimd.tensor_copy(wkq_sb, wkq_f)
            wo_f = persist.tile([128, 64], F32, tag="wof")
            nc.sync.dma_start(out=wo_f, in_=woutT_d[:, :])
            wo_sb = persist.tile([128, 64], BF16, tag="wo")
            nc.gpsimd.tensor_copy(wo_sb, wo_f)
            mask_sb = persist.tile([128, 128], F32, tag="mask")
            nc.sync.dma_start(out=mask_sb, in_=mask_d[:, :])
            permP_sb = persist.tile([128, 128], F32, tag="permP")
            nc.sync.dma_start(out=permP_sb, in_=permP_d[:, :])
            permPp_sb = persist.tile([128, 128], F32, tag="permPp")
            nc.sync.dma_start(out=permPp_sb, in_=permPp_d[:, :])
            dpat_sb = persist.tile([128, 128], F32, tag="dpat")
            nc.sync.dma_start(out=dpat_sb, in_=dpat_d[:, :])
            bo_sb = persist.tile([128, 1], F32, tag="bo")
            nc.sync.dma_start(out=bo_sb, in_=bo_d[:, :])
            lna_sb = persist.tile([128, 1], F32, tag="lna")
            nc.vector.memset(lna_sb, LN_ALPHA)
            lnla_sb = persist.tile([128, 1], F32, tag="lnla")
            nc.vector.memset(lnla_sb, LN_LAMBDA_ALPHA)
            zero_sb = persist.tile([128, 1], F32, tag="zero")
            nc.vector.memset(zero_sb, 0.0)

            # ---- x: stream fp32, cast to resident bf16 pair tiles ----
            x_sb = []
            with tc.tile_pool(name="xload", bufs=3) as xload:
                for t in range(8):
                    xf = xload.tile([128, S], F32, tag="xf")
                    nc.sync.dma_start(out=xf[0:64, :], in_=x_d[t, :, :])
                    nc.sync.dma_start(out=xf[64:128, :], in_=x_d[t + 8, :, :])
                    xb = xpool.tile([128, S], BF16, tag=f"x{t}")
                    if t % 3 == 1:
                        nc.scalar.copy(xb, xf)
                    elif t % 3 == 2:
                        nc.gpsimd.tensor_copy(xb, xf)
                    else:
                        nc.vector.tensor_copy(xb, xf)
                    x_sb.append(xb)

            # BigB result tiles (persist into phase 2), bf16 for the mix matmul
            bigB = []
            for g in range(NG):
                bigB_t = persist.tile([128, 128], BF16, tag=f"bigB{g}")
                bigB.append(bigB_t)

            # =========================== PHASE 1 ===========================
            # vscr scratch for the value-conv partition regroup (see phase 2
            # comment). The value stage is interleaved into phase 1 so its
            # matmuls/stores overlap the kq/gram pipeline.
            vscr_d = nc.dram_tensor("vscr", [NCH2, NG, 128, SC2], BF16)
            vstores = {}
            with (
                tc.tile_pool(name="p1sb", bufs=3) as p1sb,
                tc.tile_pool(name="p1sc", bufs=3) as p1sc,
                tc.tile_pool(name="vsb", bufs=3) as vsb,
                tc.tile_pool(name="kqps", bufs=2, space="PSUM") as kqps,
                tc.tile_pool(name="gramps", bufs=1, space="PSUM") as gramps,
                tc.tile_pool(name="vps", bufs=2, space="PSUM") as vps,
            ):
                gram_ps = []
                for gb in range(2):
                    gram_t = gramps.tile([128, 512], F32, tag=f"gram{gb}")
                    gram_ps.append(gram_t)

                def stage_value(pc):
                    sl = slice(SC2 * pc, SC2 * (pc + 1))
                    vdst = vscr_d[pc].rearrange(
                        "g (i2 u it) s -> it i2 g u s", i2=2, u=8, it=8)
                    stores = []
                    for t in range(8):
                        ps = vps.tile([128, SC2], F32, tag="vps")
                        nc.tensor.matmul(
                            ps[0:64, :], wv_sb[0:64, :], x_sb[t][0:64, sl],
                            start=True, stop=True)
                        nc.tensor.matmul(
                            ps[64:128, :], wv_sb[64:128, :],
                            x_sb[t][64:128, sl],
                            start=True, stop=True)
                        vpair = vsb.tile([128, SC2], BF16, tag="vpair")
                        nc.vector.tensor_copy(vpair, ps)
                        for i2 in range(2):
                            sti = nc.sync.dma_start(
                                out=vdst[t, i2],
                                in_=vpair[64 * i2: 64 * (i2 + 1), :])
                            stores.append(sti)
                    vstores[pc] = stores

                for sc in range(NCH1):
                    sl = slice(SC1 * sc, SC1 * (sc + 1))
                    # kqT free layout: half*1024 + m*64 + c   (bf16)
                    kqT = p1sb.tile([128, K * 128], BF16, tag="kqT")
                    for blk in range(2):  # member blocks [0..8), [8..16)
                        ps = kqps.tile([128, 8 * 128], F32, tag="kqps")
                        for mb in range(8):
                            m = blk * 8 + mb
                            xt = x_sb[m % 8]
                            rhalf = slice(0, 64) if m < 8 else slice(64, 128)
                            nc.tensor.matmul(
                                ps[:, 128 * mb: 128 * (mb + 1)],
                                xt[rhalf, sl], wkq_sb[rhalf, :],
                                start=True, stop=True,
                            )
                        # selu: e2 = exp(kq + ln a); r = relu(kq);
                        # out = (e2 - a) min r   (all bf16 outputs).
                        # e2/r stored half-split (h, mb, c) so the stt reads
                        # contiguous halves (DVE 2x bf16 mode).
                        e2 = p1sc.tile([128, 8 * 128], BF16, tag="e2")
                        e2v = e2.rearrange("p (h mb c) -> p mb h c",
                                           mb=8, h=2, c=64)
                        nc.scalar.activation(
                            out=e2v, in_=ps,
                            func=mybir.ActivationFunctionType.Exp,
                            bias=lna_sb[:, 0:1])
                        r = p1sc.tile([128, 8 * 128], BF16, tag="r")
                        rv = r.rearrange("p (h mb c) -> p mb h c",
                                         mb=8, h=2, c=64)
                        if (sc + blk) % 2 == 0:
                            nc.scalar.activation(
                                out=rv, in_=ps,
                                func=mybir.ActivationFunctionType.Relu,
                                bias=zero_sb[:, 0:1])
                        else:
                            nc.vector.tensor_scalar(
                                out=rv, in0=ps, scalar1=0.0, scalar2=None,
                                op0=mybir.AluOpType.max)
                        for half in range(2):
                            nc.vector.scalar_tensor_tensor(
                                out=kqT[:, 1024 * half + 512 * blk:
                                        1024 * half + 512 * (blk + 1)],
                                in0=e2[:, 512 * half: 512 * (half + 1)],
                                scalar=ALPHA,
                                in1=r[:, 512 * half: 512 * (half + 1)],
                                op0=mybir.AluOpType.subtract,
                                op1=mybir.AluOpType.min)
                    # gram: lhsT = q side (M = 8j+u), rhs = k side (N = 8i+u'),
                    # single-stride [[8,128]] APs at offset g
                    vq = kqT.rearrange("p (f e) -> p e f", f=256, e=8)
                    for g in range(NG):
                        q_ap = vq[:, g, 128:256]
                        k_ap = vq[:, g, 0:128]
                        nc.tensor.matmul(
                            gram_ps[g // 4][:, 128 * (g % 4): 128 * (g % 4 + 1)],
                            q_ap, k_ap,
                            start=(sc == 0 and g % 4 == 0),
                            stop=(sc == NCH1 - 1 and g % 4 == 3))
                    if sc % 4 == 3:
                        stage_value(sc // 4)

                # ---- softmax (no max-sub; range pre-verified) + BigB ----
                for g in range(NG):
                    gp = gram_ps[g // 4][:, 128 * (g % 4): 128 * (g % 4 + 1)]
                    E = p1sc.tile([128, 128], F32, tag="E")
                    nc.scalar.activation(
                        out=E, in_=gp,
                        func=mybir.ActivationFunctionType.Exp,
                        bias=zero_sb[:, 0:1], scale=GRAM_SCALE)
                    Ssum = p1sc.tile([128, 8], F32, tag="Ssum")
                    nc.vector.tensor_reduce(
                        out=Ssum,
                        in_=E.rearrange("p (i u) -> p u i", i=16, u=8),
                        axis=mybir.AxisListType.X, op=mybir.AluOpType.add)
                    R = p1sc.tile([128, 8], F32, tag="R")
                    nc.vector.reciprocal(out=R, in_=Ssum)
                    Eu = E.rearrange("p (i u) -> p u i", i=16, u=8)
                    for u in range(8):
                        nc.vector.tensor_scalar(
                            out=Eu[:, u, :], in0=Eu[:, u, :],
                            scalar1=R[:, u: u + 1], scalar2=None,
                            op0=mybir.AluOpType.mult)
                    nc.vector.tensor_tensor(
                        out=E, in0=E, in1=mask_sb, op=mybir.AluOpType.mult)
                    c_ps = kqps.tile([128, 128], F32, tag="kqps")
                    nc.tensor.matmul(c_ps, E, permPp_sb, start=True, stop=True)
                    c_sb = p1sc.tile([128, 128], F32, tag="permcsb")
                    nc.scalar.copy(c_sb, c_ps)
                    b_ps = kqps.tile([128, 128], F32, tag="kqps")
                    nc.tensor.matmul(b_ps, permP_sb, c_sb, start=True, stop=True)
                    nc.vector.scalar_tensor_tensor(
                        out=bigB[g], in0=b_ps, scalar=1.0, in1=dpat_sb,
                        op0=mybir.AluOpType.mult, op1=mybir.AluOpType.add)

            # =========================== PHASE 2 ===========================
            # SBUF->SBUF DMAs lower to SP-serial DIRECT2D (slow), so the
            # partition-regrouping shuffles round-trip through DRAM scratch:
            # DRAM-involved DMAs ride the fast DGE path and DRAM-side access
            # patterns may stride arbitrarily. Scratch is per-chunk (no WAR).
            # Pipelined 2 stages: mix(pc) | out(pc-1); value ran in phase 1.
            # vscr[pc][g][64*i2 + 8u + it][s] -- gather g is a plain 2D load.
            # mscr[pc][jt][64*j2 + 8g + u][s] -- tload jt is a plain 2D load.
            mscr_d = nc.dram_tensor("mscr", [NCH2, 8, 128, SC2], BF16)
            with (
                tc.tile_pool(name="p2sc", bufs=6) as p2sc,
                tc.tile_pool(name="p2out", bufs=3) as p2outp,
                tc.tile_pool(name="mps", bufs=3, space="PSUM") as mps,
                tc.tile_pool(name="ops", bufs=2, space="PSUM") as ops,
            ):
                mstores = {}

                def stage_mix(pc):
                    stores = vstores.pop(pc)
                    mdst = mscr_d[pc].rearrange(
                        "jt (j2 gg u) s -> gg j2 u jt s", j2=2, gg=8, u=8)
                    mst = []
                    for g in range(NG):
                        pm = mps.tile([128, SC2], F32, tag="mps")
                        vg = p2sc.tile([128, SC2], BF16, tag="vg")
                        gi = nc.scalar.dma_start(out=vg, in_=vscr_d[pc, g])
                        for sti in stores:
                            tile.add_dep_helper(
                                gi.ins, sti.ins, reason="gather after vstores")
                        nc.tensor.matmul(pm, bigB[g], vg, start=True, stop=True)
                        mg = p2sc.tile([128, SC2], BF16, tag="mg")
                        nc.scalar.copy(mg, pm)
                        for j2 in range(2):
                            si = nc.sync.dma_start(
                                out=mdst[g, j2],
                                in_=mg[64 * j2: 64 * (j2 + 1), :])
                            mst.append(si)
                    mstores[pc] = mst

                def stage_out(pc):
                    sl = slice(SC2 * pc, SC2 * (pc + 1))
                    mst = mstores.pop(pc)
                    for jt in range(8):
                        tpair = p2sc.tile([128, SC2], BF16, tag="tpair")
                        li = nc.scalar.dma_start(out=tpair, in_=mscr_d[pc, jt])
                        for si in mst:
                            tile.add_dep_helper(
                                li.ins, si.ins, reason="tload after mstores")
                        po = ops.tile([128, SC2], F32, tag="ops")
                        nc.tensor.matmul(
                            po[0:64, :], wo_sb[0:64, :], tpair[0:64, :],
                            start=True, stop=True)
                        nc.tensor.matmul(
                            po[64:128, :], wo_sb[64:128, :], tpair[64:128, :],
                            start=True, stop=True)
                        # y = po + b_out + x   (residual from resident bf16 x)
                        ty = p2sc.tile([128, SC2], F32, tag="ty")
                        nc.vector.scalar_tensor_tensor(
                            out=ty, in0=po, scalar=bo_sb[:, 0:1],
                            in1=x_sb[jt][:, sl],
                            op0=mybir.AluOpType.add, op1=mybir.AluOpType.add)
                        # selu(y) = min(l*a*e^y - l*a, l*relu(y))
                        e2f = p2sc.tile([128, SC2], F32, tag="fe2")
                        nc.scalar.activation(
                            out=e2f, in_=ty,
                            func=mybir.ActivationFunctionType.Exp,
                            bias=lnla_sb[:, 0:1])
                        r2f = p2sc.tile([128, SC2], F32, tag="fr2")
                        nc.vector.tensor_scalar(
                            out=r2f, in0=ty, scalar1=0.0, scalar2=LAMBDA,
                            op0=mybir.AluOpType.max, op1=mybir.AluOpType.mult)
                        o_sb = p2outp.tile([128, SC2], F32, tag="osb")
                        nc.vector.scalar_tensor_tensor(
                            out=o_sb, in0=e2f, scalar=float(LAMBDA * ALPHA),
                            in1=r2f,
                            op0=mybir.AluOpType.subtract,
                            op1=mybir.AluOpType.min)
                        nc.sync.dma_start(out=out_d[jt, pc], in_=o_sb)

                for pc in range(NCH2 + 1):
                    if pc < NCH2:
                        stage_mix(pc)
                    if pc >= 1:
                        stage_out(pc - 1)
    nc.compile()
    return nc


_NC_CACHE = None


def _get_nc():
    global _NC_CACHE
    if _NC_CACHE is None:
        _NC_CACHE = build_nc()
    return _NC_CACHE


def kernel(in_tensor, w_value, w_key, w_query, w_out, b_out, **_ignored):
    in_tensor = np.asarray(in_tensor, dtype=np.float32)
    w_value = np.asarray(w_value, dtype=np.float32)
    w_key = np.asarray(w_key, dtype=np.float32)
    w_query = np.asarray(w_query, dtype=np.float32)
    w_out = np.asarray(w_out, dtype=np.float32)
    b_out = np.asarray(b_out, dtype=np.float32)

    B = in_tensor.shape[0]
    assert B == 8
    consts = host_constants(w_value, w_key, w_query, w_out, b_out)

    nc = _get_nc()
    in_maps = []
    for b in range(B):
        m = {"x": np.ascontiguousarray(in_tensor[b].reshape(K, C, S))}
        m.update(consts)
        in_maps.append(m)

    from concourse.bass_utils import run_bass_kernel_spmd

    res = run_bass_kernel_spmd(nc, in_maps, core_ids=list(range(8)))
    outs = []
    for b in range(B):
        o = res.results[b]["out"].reshape(8, NCH2, 2, C, SC2)
        # out[jt, pc, m2, c, s] -> member (8*m2+jt), c, (pc*SC2+s)
        o = np.transpose(o, (2, 0, 3, 1, 4)).reshape(K, C, 64, 64)
        outs.append(o)
    return np.stack(outs, axis=0).astype(np.float32)


if __name__ == "__main__":
    build_nc()
    print("built ok")



# revision 9
# speedup vs baseline: 2.1997x; 1.1164x over previous
"""Trainium2 Bass kernel for nn_BaseTransformer (ensemble member-attention block).

Sharding: data-parallel over batch B=8 across 8 NeuronCores (1 batch each).
Weights/constants replicated. No collectives.

Reference math (per batch b, x = in_tensor[b] as [K=16, C=64, S=4096]):
  value = einsum('ics,oc->ios', x, Wv)
  key   = selu(einsum(x, Wk)); query = selu(einsum(x, Wq))
  gram[c,i,j] = sum_s key[i,c,s] query[j,c,s] / 64        (then * lambda^2 fold)
  A = softmax(gram, axis=i) + I
  transformed[j] = sum_i (A[c,i,j] - 1/16) value_i        (exact mean fold)
  out = selu(x + einsum(transformed, w_out) + b_out)

v3 scheme (changes vs v2):
  - Every DMA instruction costs ~0.56us of serial HWDGE sequencer time
    (DIRECT2D descriptor gen), so scratch/out transfers are batched 4x along
    the spatial dim (SCB=2048 blocks): 464 DMA insts -> ~130.
  - Output layout [jt, blk, (m2 c), (pc4 s)]: each store is one contiguous
    1MB block (stripes across all 16 SDMA engines). Host inverse-permutes.
  - No phase-2 x reload: the residual uses the resident bf16 x tiles.
  - Value conv runs as 16 bursts interleaved into the phase-1 chunk loop so
    its stores complete before the mix gathers need them.
  - Phase-1 kq PSUM in 4-member quarter blocks [128,512] (kqps bufs=4) for
    deeper cross-chunk pipelining.
"""

import sys

if "/opt/trn_rl_repo" not in sys.path:
    sys.path.insert(0, "/opt/trn_rl_repo")

import numpy as np

import concourse.bass as bass
import concourse.bacc as bacc
import concourse.mybir as mybir
import concourse.tile as tile

F32 = mybir.dt.float32
BF16 = mybir.dt.bfloat16

K, C, HEADS, S = 16, 64, 64, 4096
NG = 8           # head groups of 8 (stride-8: group g = heads {8u+g})
SC1 = 128        # phase-1 spatial chunk (gram contraction tile)
NCH1 = S // SC1  # 32
SC2 = 512        # phase-2 spatial sub-chunk (one PSUM bank)
NCH2 = S // SC2  # 8
BLK = 4          # phase-2 sub-chunks per scratch block
NBLK = NCH2 // BLK  # 2
SCB = BLK * SC2  # 2048

ALPHA = 1.6732632423543772
LAMBDA = 1.0507009873554805
LN_ALPHA = float(np.log(ALPHA))
LN_LAMBDA_ALPHA = float(np.log(LAMBDA * ALPHA))
GRAM_SCALE = float(LAMBDA * LAMBDA / 64.0)


def _pi(u, i):
    return 64 * (i // 8) + 8 * u + (i % 8)


def host_constants(w_value, w_key, w_query, w_out, b_out):
    """Build all replicated device inputs on the host."""
    consts = {}
    # sigma: head c = 8u+g  <->  storage position 8g+u (group-contiguous).
    sigma = np.zeros(64, np.int64)
    for u in range(8):
        for g in range(8):
            sigma[8 * g + u] = 8 * u + g
    wvT = np.ascontiguousarray(w_value.T[:, sigma])
    consts["wvT"] = np.concatenate([wvT, wvT], axis=0).astype(np.float32)
    wkqT = np.ascontiguousarray(np.concatenate([w_key.T, w_query.T], axis=1))
    consts["wkqT"] = np.concatenate([wkqT, wkqT], axis=0).astype(np.float32)
    woutT = np.ascontiguousarray(w_out.T[sigma, :])
    consts["woutT"] = np.concatenate([woutT, woutT], axis=0).astype(np.float32)

    # Gram psum layout: partition = 8j+u (q side), free = 8i+u' (k side).
    # MASK zeroes cross-head entries (u != u').
    mask = np.zeros((128, 128), np.float32)
    for p in range(128):
        for f in range(128):
            if p % 8 == f % 8:
                mask[p, f] = 1.0
    consts["maskg"] = mask

    # P (mm2 lhsT): rows r=(i,u)=8i+u -> out partition pi(u, i); same matrix
    # serves as P' (mm1 rhs) for the j side.
    P = np.zeros((128, 128), np.float32)
    for u in range(8):
        for i in range(16):
            P[8 * i + u, _pi(u, i)] = 1.0
    consts["permP"] = P
    consts["permPp"] = P.copy()

    # DPAT in permuted coords: D[pi(u,i), pi(u,j)] = delta(i,j) - 1/16.
    D = np.zeros((128, 128), np.float32)
    for u in range(8):
        for i in range(16):
            for j in range(16):
                D[_pi(u, i), _pi(u, j)] = (1.0 if i == j else 0.0) - 1.0 / 16.0
    consts["dpat"] = D

    consts["b_out_col"] = np.concatenate([b_out, b_out]).astype(
        np.float32).reshape(128, 1)
    return consts


def build_nc():
    """Build the single-core Bass program (same NEFF on all 8 cores)."""
    nc = bacc.Bacc("TRN2", target_bir_lowering=False, debug=False)

    x_d = nc.dram_tensor("x", [K, C, S], F32, kind="ExternalInput")
    wvT_d = nc.dram_tensor("wvT", [128, 64], F32, kind="ExternalInput")
    wkqT_d = nc.dram_tensor("wkqT", [128, 128], F32, kind="ExternalInput")
    woutT_d = nc.dram_tensor("woutT", [128, 64], F32, kind="ExternalInput")
    mask_d = nc.dram_tensor("maskg", [128, 128], F32, kind="ExternalInput")
    permP_d = nc.dram_tensor("permP", [128, 128], F32, kind="ExternalInput")
    permPp_d = nc.dram_tensor("permPp", [128, 128], F32, kind="ExternalInput")
    dpat_d = nc.dram_tensor("dpat", [128, 128], F32, kind="ExternalInput")
    bo_d = nc.dram_tensor("b_out_col", [128, 1], F32, kind="ExternalInput")
    out_d = nc.dram_tensor("out", [8, NBLK, 128, SCB], F32,
                           kind="ExternalOutput")

    with tile.TileContext(nc) as tc:
        with (
            tc.tile_pool(name="persist", bufs=1) as persist,
            tc.tile_pool(name="xpool", bufs=1) as xpool,
        ):
            # ---- weights / constants to SBUF (+ bf16 casts) ----
            wv_f = persist.tile([128, 64], F32, tag="wvf")
            nc.sync.dma_start(out=wv_f, in_=wvT_d[:, :])
            wv_sb = persist.tile([128, 64], BF16, tag="wv")
            nc.gpsimd.tensor_copy(wv_sb, wv_f)
            wkq_f = persist.tile([128, 128], F32, tag="wkqf")
            nc.sync.dma_start(out=wkq_f, in_=wkqT_d[:, :])
            wkq_sb = persist.tile([128, 128], BF16, tag="wkq")
            nc.gpsimd.tensor_copy(wkq_sb, wkq_f)
            wo_f = persist.tile([128, 64], F32, tag="wof")
            nc.sync.dma_start(out=wo_f, in_=woutT_d[:, :])
            wo_sb = persist.tile([128, 64], BF16, tag="wo")
            nc.gpsimd.tensor_copy(wo_sb, wo_f)
            mask_sb = persist.tile([128, 128], F32, tag="mask")
            nc.sync.dma_start(out=mask_sb, in_=mask_d[:, :])
            permP_sb = persist.tile([128, 128], F32, tag="permP")
            nc.sync.dma_start(out=permP_sb, in_=permP_d[:, :])
            permPp_sb = persist.tile([128, 128], F32, tag="permPp")
            nc.sync.dma_start(out=permPp_sb, in_=permPp_d[:, :])
            dpat_sb = persist.tile([128, 128], F32, tag="dpat")
            nc.sync.dma_start(out=dpat_sb, in_=dpat_d[:, :])
            bo_sb = persist.tile([128, 1], F32, tag="bo")
            nc.sync.dma_start(out=bo_sb, in_=bo_d[:, :])
            lna_sb = persist.tile([128, 1], F32, tag="lna")
            nc.vector.memset(lna_sb, LN_ALPHA)
            lnla_sb = persist.tile([128, 1], F32, tag="lnla")
            nc.vector.memset(lnla_sb, LN_LAMBDA_ALPHA)
            zero_sb = persist.tile([128, 1], F32, tag="zero")
            nc.vector.memset(zero_sb, 0.0)

            # ---- x: stream fp32, cast to resident bf16 pair tiles ----
            x_sb = []
            with tc.tile_pool(name="xload", bufs=3) as xload:
                for t in range(8):
                    xf = xload.tile([128, S], F32, tag="xf")
                    nc.sync.dma_start(out=xf[0:64, :], in_=x_d[t, :, :])
                    nc.sync.dma_start(out=xf[64:128, :], in_=x_d[t + 8, :, :])
                    xb = xpool.tile([128, S], BF16, tag=f"x{t}")
                    if t % 2 == 1:
                        nc.gpsimd.tensor_copy(xb, xf)
                    else:
                        nc.vector.tensor_copy(xb, xf)
                    x_sb.append(xb)

            # BigB result tiles (persist into phase 2), bf16 for the mix matmul
            bigB = []
            for g in range(NG):
                bigB_t = persist.tile([128, 128], BF16, tag=f"bigB{g}")
                bigB.append(bigB_t)

            # =========================== PHASE 1 ===========================
            # vscr[blk][g][64*i2 + 8u + it][s'] -- gather (blk,g) is one
            # contiguous 512KB load; s' spans BLK=4 phase-2 sub-chunks.
            vscr_d = nc.dram_tensor("vscr", [NBLK, NG, 128, SCB], BF16)
            vstores = {0: [], 1: []}
            with (
                tc.tile_pool(name="p1sb", bufs=3) as p1sb,
                tc.tile_pool(name="p1sc", bufs=3) as p1sc,
                tc.tile_pool(name="vsb", bufs=2) as vsb,
                tc.tile_pool(name="kqps", bufs=4, space="PSUM") as kqps,
                tc.tile_pool(name="gramps", bufs=1, space="PSUM") as gramps,
                tc.tile_pool(name="vps", bufs=2, space="PSUM") as vps,
            ):
                gram_ps = []
                for gb in range(2):
                    gram_t = gramps.tile([128, 512], F32, tag=f"gram{gb}")
                    gram_ps.append(gram_t)

                def stage_value_t(blk, t):
                    # value conv for member pair t over one SCB block:
                    # 4 matmul pairs -> bf16 assembly tile -> 2 batched stores
                    vblk = vsb.tile([128, SCB], BF16, tag="vblk")
                    for q4 in range(BLK):
                        sl = slice(SCB * blk + SC2 * q4,
                                   SCB * blk + SC2 * (q4 + 1))
                        ps = vps.tile([128, SC2], F32, tag="vps")
                        nc.tensor.matmul(
                            ps[0:64, :], wv_sb[0:64, :], x_sb[t][0:64, sl],
                            start=True, stop=True)
                        nc.tensor.matmul(
                            ps[64:128, :], wv_sb[64:128, :],
                            x_sb[t][64:128, sl],
                            start=True, stop=True)
                        dst = vblk[:, SC2 * q4: SC2 * (q4 + 1)]
                        if (t + q4) % 2 == 0:
                            nc.vector.tensor_copy(dst, ps)
                        else:
                            nc.scalar.copy(dst, ps)
                    vdst = vscr_d[blk].rearrange(
                        "g (i2 u it) s -> it i2 g u s", i2=2, u=8, it=8)
                    for i2 in range(2):
                        sti = nc.sync.dma_start(
                            out=vdst[t, i2],
                            in_=vblk[64 * i2: 64 * (i2 + 1), :])
                        vstores[blk].append(sti)

                for sc in range(NCH1):
                    sl = slice(SC1 * sc, SC1 * (sc + 1))
                    # kqT free layout: half*1024 + m*64 + c   (bf16)
                    kqT = p1sb.tile([128, K * 128], BF16, tag="kqT")
                    for qb in range(4):  # quarter blocks of 4 members
                        ps = kqps.tile([128, 4 * 128], F32, tag="kqps")
                        for mb in range(4):
                            m = qb * 4 + mb
                            xt = x_sb[m % 8]
                            rhalf = slice(0, 64) if m < 8 else slice(64, 128)
                            nc.tensor.matmul(
                                ps[:, 128 * mb: 128 * (mb + 1)],
                                xt[rhalf, sl], wkq_sb[rhalf, :],
                                start=True, stop=True,
                            )
                        # selu: e2 = exp(kq + ln a); r = relu(kq);
                        # out = (e2 - a) min r   (all bf16 outputs).
                        # e2/r stored half-split (h, mb, c) so the stt reads
                        # contiguous halves (DVE 2x bf16 mode).
                        e2 = p1sc.tile([128, 4 * 128], BF16, tag="e2")
                        e2v = e2.rearrange("p (h mb c) -> p mb h c",
                                           mb=4, h=2, c=64)
                        nc.scalar.activation(
                            out=e2v, in_=ps,
                            func=mybir.ActivationFunctionType.Exp,
                            bias=lna_sb[:, 0:1])
                        r = p1sc.tile([128, 4 * 128], BF16, tag="r")
                        rv = r.rearrange("p (h mb c) -> p mb h c",
                                         mb=4, h=2, c=64)
                        if (sc + qb) % 2 == 0:
                            nc.scalar.activation(
                                out=rv, in_=ps,
                                func=mybir.ActivationFunctionType.Relu,
                                bias=zero_sb[:, 0:1])
                        else:
                            nc.vector.tensor_scalar(
                                out=rv, in0=ps, scalar1=0.0, scalar2=None,
                                op0=mybir.AluOpType.max)
                        for half in range(2):
                            nc.vector.scalar_tensor_tensor(
                                out=kqT[:, 1024 * half + 256 * qb:
                                        1024 * half + 256 * (qb + 1)],
                                in0=e2[:, 256 * half: 256 * (half + 1)],
                                scalar=ALPHA,
                                in1=r[:, 256 * half: 256 * (half + 1)],
                                op0=mybir.AluOpType.subtract,
                                op1=mybir.AluOpType.min)
                    # gram: lhsT = q side (M = 8j+u), rhs = k side (N = 8i+u'),
                    # single-stride [[8,128]] APs at offset g
                    vq = kqT.rearrange("p (f e) -> p e f", f=256, e=8)
                    for g in range(NG):
                        q_ap = vq[:, g, 128:256]
                        k_ap = vq[:, g, 0:128]
                        nc.tensor.matmul(
                            gram_ps[g // 4][:, 128 * (g % 4): 128 * (g % 4 + 1)],
                            q_ap, k_ap,
                            start=(sc == 0 and g % 4 == 0),
                            stop=(sc == NCH1 - 1 and g % 4 == 3))
                    if 14 <= sc < 30:
                        bt = sc - 14
                        stage_value_t(bt // 8, bt % 8)

                # ---- softmax (no max-sub; range pre-verified) + BigB ----
                for g in range(NG):
                    gp = gram_ps[g // 4][:, 128 * (g % 4): 128 * (g % 4 + 1)]
                    E = p1sc.tile([128, 128], F32, tag="E")
                    nc.scalar.activation(
                        out=E, in_=gp,
                        func=mybir.ActivationFunctionType.Exp,
                        bias=zero_sb[:, 0:1], scale=GRAM_SCALE)
                    Ssum = p1sc.tile([128, 8], F32, tag="Ssum")
                    nc.vector.tensor_reduce(
                        out=Ssum,
                        in_=E.rearrange("p (i u) -> p u i", i=16, u=8),
                        axis=mybir.AxisListType.X, op=mybir.AluOpType.add)
                    R = p1sc.tile([128, 8], F32, tag="R")
                    nc.vector.reciprocal(out=R, in_=Ssum)
                    Eu = E.rearrange("p (i u) -> p u i", i=16, u=8)
                    for u in range(8):
                        nc.vector.tensor_scalar(
                            out=Eu[:, u, :], in0=Eu[:, u, :],
                            scalar1=R[:, u: u + 1], scalar2=None,
                            op0=mybir.AluOpType.mult)
                    nc.vector.tensor_tensor(
                        out=E, in0=E, in1=mask_sb, op=mybir.AluOpType.mult)
                    c_ps = kqps.tile([128, 128], F32, tag="kqps")
                    nc.tensor.matmul(c_ps, E, permPp_sb, start=True, stop=True)
                    c_sb = p1sc.tile([128, 128], F32, tag="permcsb")
                    nc.scalar.copy(c_sb, c_ps)
                    b_ps = kqps.tile([128, 128], F32, tag="kqps")
                    nc.tensor.matmul(b_ps, permP_sb, c_sb, start=True, stop=True)
                    nc.vector.scalar_tensor_tensor(
                        out=bigB[g], in0=b_ps, scalar=1.0, in1=dpat_sb,
                        op0=mybir.AluOpType.mult, op1=mybir.AluOpType.add)

            # =========================== PHASE 2 ===========================
            # mscr[blk][jt][64*j2 + 8g + u][s'] -- tload (blk,jt) is one
            # contiguous 512KB load.
            mscr_d = nc.dram_tensor("mscr", [NBLK, 8, 128, SCB], BF16)
            with (
                tc.tile_pool(name="p2sc", bufs=3) as p2sc,
                tc.tile_pool(name="p2out", bufs=2) as p2outp,
                tc.tile_pool(name="mps", bufs=4, space="PSUM") as mps,
                tc.tile_pool(name="ops", bufs=3, space="PSUM") as ops,
            ):
                mstores = {0: [], 1: []}

                def stage_mix(blk):
                    mdst = mscr_d[blk].rearrange(
                        "jt (j2 gg u) s -> gg j2 u jt s", j2=2, gg=8, u=8)
                    for g in range(NG):
                        vg = p2sc.tile([128, SCB], BF16, tag="vg")
                        gi = nc.scalar.dma_start(out=vg, in_=vscr_d[blk, g])
                        for sti in vstores[blk]:
                            tile.add_dep_helper(
                                gi.ins, sti.ins, reason="gather after vstores")
                        mg = p2sc.tile([128, SCB], BF16, tag="mg")
                        for q4 in range(BLK):
                            pm = mps.tile([128, SC2], F32, tag="mps")
                            nc.tensor.matmul(
                                pm, bigB[g],
                                vg[:, SC2 * q4: SC2 * (q4 + 1)],
                                start=True, stop=True)
                            dst = mg[:, SC2 * q4: SC2 * (q4 + 1)]
                            if (g + q4) % 2 == 0:
                                nc.scalar.copy(dst, pm)
                            else:
                                nc.vector.tensor_copy(dst, pm)
                        for j2 in range(2):
                            si = nc.sync.dma_start(
                                out=mdst[g, j2],
                                in_=mg[64 * j2: 64 * (j2 + 1), :])
                            mstores[blk].append(si)

                def stage_out(blk):
                    for jt in range(8):
                        tpair = p2sc.tile([128, SCB], BF16, tag="tpair")
                        li = nc.scalar.dma_start(
                            out=tpair, in_=mscr_d[blk, jt])
                        for si in mstores[blk]:
                            tile.add_dep_helper(
                                li.ins, si.ins, reason="tload after mstores")
                        oblk = p2outp.tile([128, SCB], F32, tag="osb")
                        for q4 in range(BLK):
                            sl = slice(SCB * blk + SC2 * q4,
                                       SCB * blk + SC2 * (q4 + 1))
                            tsl = slice(SC2 * q4, SC2 * (q4 + 1))
                            po = ops.tile([128, SC2], F32, tag="ops")
                            nc.tensor.matmul(
                                po[0:64, :], wo_sb[0:64, :],
                                tpair[0:64, tsl],
                                start=True, stop=True)
                            nc.tensor.matmul(
                                po[64:128, :], wo_sb[64:128, :],
                                tpair[64:128, tsl],
                                start=True, stop=True)
                            # y = po + b_out + x  (residual from bf16 x)
                            ty = p2sc.tile([128, SC2], F32, tag="ty")
                            nc.vector.scalar_tensor_tensor(
                                out=ty, in0=po, scalar=bo_sb[:, 0:1],
                                in1=x_sb[jt][:, sl],
                                op0=mybir.AluOpType.add,
                                op1=mybir.AluOpType.add)
                            # selu(y) = min(l*a*e^y - l*a, l*relu(y))
                            e2f = p2sc.tile([128, SC2], F32, tag="fe2")
                            nc.scalar.activation(
                                out=e2f, in_=ty,
                                func=mybir.ActivationFunctionType.Exp,
                                bias=lnla_sb[:, 0:1])
                            r2f = p2sc.tile([128, SC2], F32, tag="fr2")
                            nc.vector.tensor_scalar(
                                out=r2f, in0=ty, scalar1=0.0, scalar2=LAMBDA,
                                op0=mybir.AluOpType.max,
                                op1=mybir.AluOpType.mult)
                            nc.vector.scalar_tensor_tensor(
                                out=oblk[:, tsl], in0=e2f,
                                scalar=float(LAMBDA * ALPHA),
                                in1=r2f,
                                op0=mybir.AluOpType.subtract,
                                op1=mybir.AluOpType.min)
                        nc.sync.dma_start(out=out_d[jt, blk], in_=oblk)

                stage_mix(0)
                stage_mix(1)
                stage_out(0)
                stage_out(1)
    nc.compile()
    return nc


_NC_CACHE = None


def _get_nc():
    global _NC_CACHE
    if _NC_CACHE is None:
        _NC_CACHE = build_nc()
    return _NC_CACHE


def kernel(in_tensor, w_value, w_key, w_query, w_out, b_out, **_ignored):
    in_tensor = np.asarray(in_tensor, dtype=np.float32)
    w_value = np.asarray(w_value, dtype=np.float32)
    w_key = np.asarray(w_key, dtype=np.float32)
    w_query = np.asarray(w_query, dtype=np.float32)
    w_out = np.asarray(w_out, dtype=np.float32)
    b_out = np.asarray(b_out, dtype=np.float32)

    B = in_tensor.shape[0]
    assert B == 8
    consts = host_constants(w_value, w_key, w_query, w_out, b_out)

    nc = _get_nc()
    in_maps = []
    for b in range(B):
        m = {"x": np.ascontiguousarray(in_tensor[b].reshape(K, C, S))}
        m.update(consts)
        in_maps.append(m)

    from concourse.bass_utils import run_bass_kernel_spmd

    res = run_bass_kernel_spmd(nc, in_maps, core_ids=list(range(8)))
    outs = []
    for b in range(B):
        o = res.results[b]["out"].reshape(8, NBLK, 2, C, BLK, SC2)
        # out[jt, blk, m2, c, q4, s] -> member (8*m2+jt), c, blk*SCB+q4*SC2+s
        o = np.transpose(o, (2, 0, 3, 1, 4, 5)).reshape(K, C, 64, 64)
        outs.append(o)
    return np.stack(outs, axis=0).astype(np.float32)


if __name__ == "__main__":
    build_nc()
    print("built ok")
